# revision 20
# baseline (speedup 1.0000x reference)
"""AsyncCrossModalConsistencyLoss distributed Bass kernel for 8 TRN2 NeuronCores.

Data-parallel: batch dim (B=8) sharded one element per core. Each core:
  - casts its [4096, 512] visual/audio shard f32->bf16 during the DMA
    (SWDGE, 1 MB bulk chunks + 0.75/0.25 MB end taper; HBM-bound at an
    effective ~338 GB/s -> ~50 us span)
  - per [128,512] tile: row sum-of-squares on per-engine accumulators
    (ScalarE owns v squares, VectorE owns a squares — a shared tile
    serializes the engines on WAW order), v*a products (VectorE),
    1/max(norm,eps) (Sqrt + reciprocal in f32), then TensorE matmuls
    accumulate sum_s v_hat, sum_s a_hat and the sync dot-sum in PSUM
  - compute chunks taper 4,...,4,2,1,1 and the last chunk emits its whole
    v-side chain (square, norm, sumv matmul, PSUM->SBUF copy of sumv)
    before the a-side ops, so in-order engines run it during the final
    0.25 MB a-tile DMA and only the a-chain + 2 matmuls + epilogue trail
    the last byte
  - epilogue: <sumv_sb, suma_ps> dot and the scaled sync reduce run on
    DVE/ScalarE in parallel; margin/relu/target-select fold into 2 Relu
    activations + 1 Identity blend (tensor scale/bias), pre-scaled by 1/8
Each core writes its partial loss; the host sums the 8 partials.

Measured (For_i differential wall-clock, min-statistic, barrier-corrected):
~57.9 us vs ~61.1 us for the previous version; DMA-only floor ~51.6 us.
"""

import contextlib

import numpy as np

import concourse.bass as bass
import concourse.tile as tile
from concourse import bacc, mybir
from concourse.bass_utils import run_bass_kernel_spmd

N_CORES = 8
S = 4096
D = 512
P = 128
NT = S // P              # 32 compute tiles of [128, 512]
FREE = NT * D            # 16384 columns per partition
TILES_PER_CHUNK = 4          # DMA chunk (1 MB per tensor per chunk)
NCH = NT // TILES_PER_CHUNK
CHUNK_COLS = TILES_PER_CHUNK * D

# Compute chunks: bulk of 4 tiles per norm-chain batch, tapering to 1 so the
# critical path after the last DMA completion is one tile's worth of work.
COMPUTE_CHUNKS = [(0, 8), (8, 8), (16, 8), (24, 4),
                  (28, 2), (30, 1), (31, 1)]

# DMA chunks (per tensor, in issue order v,a alternating): uniform 1 MB bulk,
# then 0.75 MB + 0.25 MB at the end so only the final 1-tile compute chunk
# trails the last byte. The end-taper descgen lands when Q7 is idle, so the
# extra dma_start is off the critical path (unlike front-taper, measured
# +6 us/iter by the previous session).
DMA_CHUNKS = [(0, 4), (4, 4), (8, 4), (12, 4), (16, 4), (20, 4), (24, 4),
              (28, 3), (31, 1)]

EPS_DIV = 1e-8
MARGIN = 0.5
C_SYNC = 1.0 / S
C_ASYNC = 1.0 / (S * (S - 1) + EPS_DIV)

F32 = mybir.dt.float32
BF16 = mybir.dt.bfloat16
AF = mybir.ActivationFunctionType
OP = mybir.AluOpType


def _build(collective=False, reps=1):
    nc = bacc.Bacc(
        "TRN2", target_bir_lowering=False, debug=False,
        num_devices=N_CORES if collective else 1,
    )
    v_ext = nc.dram_tensor("v", [S, D], F32, kind="ExternalInput")
    a_ext = nc.dram_tensor("a", [S, D], F32, kind="ExternalInput")
    w_ext = nc.dram_tensor("w", [1, 1], F32, kind="ExternalInput")
    out_ext = nc.dram_tensor("out", [1, 1], F32, kind="ExternalOutput")

    # Row s = p*NT + n lands on partition p, tile n: contiguous 64KB per
    # partition in DRAM -> ideal DMA pattern. Any row->(p,n) bijection works
    # because every reduction here is symmetric over rows.
    v_re = v_ext.ap().rearrange("(p n) d -> p (n d)", p=P)
    a_re = a_ext.ap().rearrange("(p n) d -> p (n d)", p=P)

    with tile.TileContext(nc) as tc:
        with (
            tc.tile_pool(name="big", bufs=1) as big,
            tc.tile_pool(name="scratch", bufs=3) as scratch,
            tc.tile_pool(name="small", bufs=6) as small,
            tc.tile_pool(name="psum", bufs=1, space="PSUM") as psum,
            tc.tile_pool(name="dram", bufs=1, space="DRAM") as dram,
        ):
            v_sb = big.tile([P, FREE], BF16)
            a_sb = big.tile([P, FREE], BF16)
            w_sb = big.tile([1, 1], F32)
            eps_b = big.tile([P, 1], F32)
            nc.vector.memset(eps_b[:], 1e-24)
            b_r0 = big.tile([1, 1], F32)
            nc.vector.memset(b_r0[:], MARGIN / N_CORES)
            b_r1 = big.tile([1, 1], F32)
            nc.vector.memset(b_r1[:], MARGIN * 0.1 / N_CORES)
            nc.sync.dma_start(w_sb[:], w_ext[:])
            # reps>1 wraps the body in a HW loop for differential wall-clock
            # timing (repmeasure.py); the graded path is reps=1.
            loop = tc.For_i(0, reps) if reps > 1 else contextlib.nullcontext()
            with loop:
                _body(nc, scratch, small, psum, dram,
                      v_sb, a_sb, w_sb, eps_b, b_r0, b_r1,
                      v_re, a_re, out_ext, collective)

    nc.compile()
    return nc


def _body(nc, scratch, small, psum, dram,
          v_sb, a_sb, w_sb, eps_b, b_r0, b_r1,
          v_re, a_re, out_ext, collective, dma_chunks=None):
    for t0, tpc in (dma_chunks or DMA_CHUNKS):
        sl = slice(t0 * D, (t0 + tpc) * D)
        # gpsimd (SWDGE) DMA casts f32 -> bf16 in flight
        nc.gpsimd.dma_start(v_sb[:, sl], v_re[:, sl])
        nc.gpsimd.dma_start(a_sb[:, sl], a_re[:, sl])

    sumv_ps = psum.tile([1, D], F32)
    suma_ps = psum.tile([1, D], F32)
    sync_ps = psum.tile([1, D], F32)
    sumv_sb = small.tile([1, D], F32)

    n_chunks = len(COMPUTE_CHUNKS)
    for c, (t0, tpc) in enumerate(COMPUTE_CHUNKS):
        first = c == 0
        last = c == n_chunks - 1
        # Per-engine sum-of-squares accumulators: ScalarE owns ss_v, DVE
        # owns ss_a. Sharing one tile serializes the engines on WAW order.
        ss_v = small.tile([P, tpc], F32)
        ss_a = small.tile([P, tpc], F32)

        def vsq(j):
            sl = slice((t0 + j) * D, (t0 + j + 1) * D)
            sq_v = scratch.tile([P, D], BF16)
            nc.scalar.activation(
                sq_v[:], v_sb[:, sl], AF.Square, accum_out=ss_v[:, j:j + 1]
            )

        def asq(j):
            sl = slice((t0 + j) * D, (t0 + j + 1) * D)
            # DVE square-reduce via scalar_tensor_tensor accum
            # (InstTensorTensorReduce faults on this HW)
            sq_a = scratch.tile([P, D], BF16)
            nc.vector.scalar_tensor_tensor(
                out=sq_a[:], in0=a_sb[:, sl], scalar=1.0, in1=a_sb[:, sl],
                op0=OP.mult, op1=OP.mult,
                accum_out=ss_a[:, j:j + 1],
            )
            # prod = v*a (bf16 2x mode); its weighted row-sum goes
            # through the PE below, so no per-row dot accum is needed
            prod = scratch.tile([P, D], BF16, tag=f"prod{j}")
            nc.vector.tensor_tensor(
                out=prod[:], in0=v_sb[:, sl], in1=a_sb[:, sl], op=OP.mult
            )
            return prod

        # Batched 1/max(norm, eps) per tensor. The sqrt bias keeps
        # sqrt(0) finite, matching F.normalize's max(norm, 1e-12) for
        # all realizable inputs.
        def vnorm():
            nrm_v = small.tile([P, tpc], F32)
            nc.scalar.activation(nrm_v[:], ss_v[:], AF.Sqrt, bias=eps_b[:])
            inv_v = small.tile([P, tpc], F32)
            nc.vector.reciprocal(inv_v[:], nrm_v[:])
            inv_vb = small.tile([P, tpc], BF16)
            nc.scalar.copy(inv_vb[:], inv_v[:])
            return inv_v, inv_vb

        def anorm(inv_v):
            nrm_a = small.tile([P, tpc], F32)
            nc.scalar.activation(nrm_a[:], ss_a[:], AF.Sqrt, bias=eps_b[:])
            inv_a = small.tile([P, tpc], F32)
            nc.vector.reciprocal(inv_a[:], nrm_a[:])
            inv_ab = small.tile([P, tpc], BF16)
            nc.scalar.copy(inv_ab[:], inv_a[:])
            invva_b = small.tile([P, tpc], BF16)
            nc.vector.tensor_mul(invva_b[:], inv_v[:], inv_a[:])
            return inv_ab, invva_b

        def mm(ps, wcol, rhs, st, sp):
            nc.tensor.matmul(ps[:], lhsT=wcol, rhs=rhs, start=st, stop=sp)

        if not last:
            prods = []
            for j in range(tpc):
                vsq(j)
                prods.append(asq(j))
            inv_v, inv_vb = vnorm()
            inv_ab, invva_b = anorm(inv_v)
            for j in range(tpc):
                sl = slice((t0 + j) * D, (t0 + j + 1) * D)
                st = first and j == 0
                # suma first: its epilogue consumer (PSUM->SBUF copy on
                # ScalarE) can start two matmuls before sync's reduce
                mm(suma_ps, inv_ab[:, j:j + 1], a_sb[:, sl], st, False)
                mm(sumv_ps, inv_vb[:, j:j + 1], v_sb[:, sl], st, False)
                mm(sync_ps, invva_b[:, j:j + 1], prods[j][:], st, False)
        else:
            # Last chunk (1 tile): the entire v-side chain (square, norm,
            # sumv matmul, PSUM->SBUF copy of the finished sumv) is emitted
            # first so in-order engines run it during the final a-tile's
            # DMA; only the a-side chain and two matmuls trail the last
            # byte, and the epilogue dot reads suma from PSUM directly.
            sl = slice(t0 * D, (t0 + tpc) * D)
            vsq(0)
            inv_v, inv_vb = vnorm()
            mm(sumv_ps, inv_vb[:, 0:1], v_sb[:, sl], False, True)
            nc.scalar.copy(sumv_sb[:], sumv_ps[:])
            prod = asq(0)
            inv_ab, invva_b = anorm(inv_v)
            mm(suma_ps, inv_ab[:, 0:1], a_sb[:, sl], False, True)
            mm(sync_ps, invva_b[:, 0:1], prod[:], False, True)

    # ---- epilogue: scalars on partition 0 ----
    # tot = C_ASYNC * <suma (PSUM), sumv_sb>   (DVE; sumv was copied to
    # SBUF during the final a-DMA, so no PSUM->SBUF copy sits here)
    # syn = -(C_SYNC + C_ASYNC) * sum(sync_ps)   (ScalarE, in parallel)
    # diff = tot + syn = async_mean - sync_mean
    # loss/8 = r1 + w*(r0 - r1), r0/r1 relu branches pre-scaled by 1/8
    tot = small.tile([1, 1], F32)
    dum = small.tile([1, D], F32)
    nc.vector.scalar_tensor_tensor(
        out=dum[:], in0=suma_ps[:], scalar=C_ASYNC, in1=sumv_sb[:],
        op0=OP.mult, op1=OP.mult, accum_out=tot[:],
    )
    syn = small.tile([1, 1], F32)
    dum2 = small.tile([1, D], F32)
    nc.scalar.activation(
        dum2[:], sync_ps[:], AF.Copy, scale=-(C_SYNC + C_ASYNC),
        accum_out=syn[:],
    )
    diff = small.tile([1, 1], F32)
    nc.vector.tensor_add(diff[:], tot[:], syn[:])
    # Both margin branches, relu, and the target blend stay on DVE: the
    # tot/diff chain is already there, so no cross-engine hops remain in
    # the tail (each hop costs ~0.15 us of semaphore latency).
    marg = small.tile([1, 2], F32)
    nc.vector.tensor_scalar(
        marg[:, 0:1], diff[:], 1.0 / N_CORES, MARGIN / N_CORES,
        op0=OP.mult, op1=OP.add,
    )
    nc.vector.tensor_scalar(
        marg[:, 1:2], diff[:], -1.0 / N_CORES, MARGIN * 0.1 / N_CORES,
        op0=OP.mult, op1=OP.add,
    )
    relu = small.tile([1, 2], F32)
    nc.vector.tensor_scalar_max(relu[:], marg[:], 0.0)
    d01 = small.tile([1, 1], F32)
    nc.vector.tensor_sub(d01[:], relu[:, 0:1], relu[:, 1:2])
    wd = small.tile([1, 1], F32)
    nc.vector.tensor_mul(wd[:], d01[:], w_sb[:])
    lscaled = small.tile([1, 1], F32)
    nc.vector.tensor_add(lscaled[:], wd[:], relu[:, 1:2])

    if collective:
        loss_bounce = dram.tile([1, 1], F32)
        out_bounce = dram.tile([1, 1], F32)
        nc.gpsimd.dma_start(loss_bounce[:], lscaled[:])
        nc.gpsimd.collective_compute(
            "AllReduce",
            OP.add,
            replica_groups=[list(range(N_CORES))],
            ins=[loss_bounce.opt()],
            outs=[out_bounce.opt()],
        )
        nc.gpsimd.dma_start(out_ext[:], out_bounce[:])
    else:
        nc.sync.dma_start(out_ext[:], lscaled[:])


_NC = None


def _get_nc():
    global _NC
    if _NC is None:
        _NC = _build()
    return _NC


def make_in_maps(visual_features, audio_features, targets):
    vf = np.asarray(visual_features)
    af = np.asarray(audio_features)
    tg = np.asarray(targets)
    return [
        {
            "v": np.ascontiguousarray(vf[i], dtype=np.float32),
            "a": np.ascontiguousarray(af[i], dtype=np.float32),
            "w": np.array([[float(tg[i])]], dtype=np.float32),
        }
        for i in range(N_CORES)
    ]


def kernel(visual_features, audio_features, targets):
    nc = _get_nc()
    in_maps = make_in_maps(visual_features, audio_features, targets)
    res = run_bass_kernel_spmd(nc, in_maps, core_ids=list(range(N_CORES)))
    # Each core's out is its batch element's loss pre-scaled by 1/8; the
    # global mean is the sum of the 8 partials.
    total = np.float32(0.0)
    for i in range(N_CORES):
        total += np.asarray(res.results[i]["out"], dtype=np.float32).reshape(())
    return np.float32(total)


if __name__ == "__main__":
    rng = np.random.default_rng(0)
    v = rng.standard_normal((N_CORES, S, D)).astype(np.float32)
    a = rng.standard_normal((N_CORES, S, D)).astype(np.float32)
    t = rng.integers(0, 2, (N_CORES,)).astype(np.int32)
    print(kernel(visual_features=v, audio_features=a, targets=t))


# revision 21
# speedup vs baseline: 1.0070x; 1.0070x over previous
"""AsyncCrossModalConsistencyLoss distributed Bass kernel for 8 TRN2 NeuronCores.

Data-parallel: batch dim (B=8) sharded one element per core. Each core:
  - casts its [4096, 512] visual/audio shard f32->bf16 during the DMA
    (SWDGE, 1 MB bulk chunks + 0.75/0.25 MB end taper; HBM-bound at an
    effective ~338 GB/s -> ~50 us span)
  - per [128,512] tile: row sum-of-squares on per-engine accumulators
    (ScalarE owns v squares, VectorE owns a squares — a shared tile
    serializes the engines on WAW order), v*a products (VectorE),
    1/max(norm,eps) (Sqrt + reciprocal in f32), then TensorE matmuls
    accumulate sum_s v_hat, sum_s a_hat and the sync dot-sum in PSUM
  - compute chunks 8,8,8,4 then taper 2,1,1 (few norm-chain instances in
    steady state); the last chunk emits its whole v-side chain (square,
    norm, sumv matmul, PSUM->SBUF copy of sumv) before the a-side ops, so
    in-order engines run it during the final 0.25 MB a-tile DMA and only
    the a-chain + 2 matmuls + epilogue trail the last byte
  - epilogue: <sumv_sb, suma_ps> dot and the scaled sync reduce run on
    DVE/ScalarE in parallel; margin/relu/target-select then stay entirely
    on DVE (pre-scaled by 1/8) so no cross-engine hops remain in the tail
Each core writes its partial loss; the host sums the 8 partials.

Measured (For_i differential wall-clock, min-statistic, barrier-corrected):
~57.5 us vs ~61.4 us for the staged baseline; DMA-only floor ~50.7 us
(338 GB/s effective, insensitive to chunk size and DGE path).
"""

import contextlib

import numpy as np

import concourse.bass as bass
import concourse.tile as tile
from concourse import bacc, mybir
from concourse.bass_utils import run_bass_kernel_spmd

N_CORES = 8
S = 4096
D = 512
P = 128
NT = S // P              # 32 compute tiles of [128, 512]
FREE = NT * D            # 16384 columns per partition
TILES_PER_CHUNK = 4          # DMA chunk (1 MB per tensor per chunk)
NCH = NT // TILES_PER_CHUNK
CHUNK_COLS = TILES_PER_CHUNK * D

# Compute chunks: 8-tile bulk batches per norm chain, tapering to 1 so the
# critical path after the last DMA completion is one tile's worth of work.
COMPUTE_CHUNKS = [(0, 8), (8, 8), (16, 8), (24, 4),
                  (28, 2), (30, 1), (31, 1)]

# DMA chunks (per tensor, in issue order v,a alternating): uniform 1 MB bulk,
# then 0.75 MB + 0.25 MB at the end so only the final 1-tile compute chunk
# trails the last byte. The end-taper descgen lands when Q7 is idle, so the
# extra dma_start is off the critical path (unlike front-taper, measured
# +6 us/iter by the previous session).
DMA_CHUNKS = [(0, 4), (4, 4), (8, 4), (12, 4), (16, 4), (20, 4), (24, 4),
              (28, 3), (31, 1)]

EPS_DIV = 1e-8
MARGIN = 0.5
C_SYNC = 1.0 / S
C_ASYNC = 1.0 / (S * (S - 1) + EPS_DIV)

F32 = mybir.dt.float32
BF16 = mybir.dt.bfloat16
AF = mybir.ActivationFunctionType
OP = mybir.AluOpType


def _build(collective=False, reps=1):
    nc = bacc.Bacc(
        "TRN2", target_bir_lowering=False, debug=False,
        num_devices=N_CORES if collective else 1,
    )
    v_ext = nc.dram_tensor("v", [S, D], F32, kind="ExternalInput")
    a_ext = nc.dram_tensor("a", [S, D], F32, kind="ExternalInput")
    w_ext = nc.dram_tensor("w", [1, 1], F32, kind="ExternalInput")
    out_ext = nc.dram_tensor("out", [1, 1], F32, kind="ExternalOutput")

    # Row s = p*NT + n lands on partition p, tile n: contiguous 64KB per
    # partition in DRAM -> ideal DMA pattern. Any row->(p,n) bijection works
    # because every reduction here is symmetric over rows.
    v_re = v_ext.ap().rearrange("(p n) d -> p (n d)", p=P)
    a_re = a_ext.ap().rearrange("(p n) d -> p (n d)", p=P)

    with tile.TileContext(nc) as tc:
        with (
            tc.tile_pool(name="big", bufs=1) as big,
            tc.tile_pool(name="scratch", bufs=3) as scratch,
            tc.tile_pool(name="small", bufs=6) as small,
            tc.tile_pool(name="psum", bufs=1, space="PSUM") as psum,
            tc.tile_pool(name="dram", bufs=1, space="DRAM") as dram,
        ):
            v_sb = big.tile([P, FREE], BF16)
            a_sb = big.tile([P, FREE], BF16)
            w_sb = big.tile([1, 1], F32)
            eps_b = big.tile([P, 1], F32)
            nc.vector.memset(eps_b[:], 1e-24)
            b_r0 = big.tile([1, 1], F32)
            nc.vector.memset(b_r0[:], MARGIN / N_CORES)
            b_r1 = big.tile([1, 1], F32)
            nc.vector.memset(b_r1[:], MARGIN * 0.1 / N_CORES)
            nc.sync.dma_start(w_sb[:], w_ext[:])
            # reps>1 wraps the body in a HW loop for differential wall-clock
            # timing (repmeasure.py); the graded path is reps=1.
            loop = tc.For_i(0, reps) if reps > 1 else contextlib.nullcontext()
            with loop:
                _body(nc, scratch, small, psum, dram,
                      v_sb, a_sb, w_sb, eps_b, b_r0, b_r1,
                      v_re, a_re, out_ext, collective)

    nc.compile()
    return nc


def _body(nc, scratch, small, psum, dram,
          v_sb, a_sb, w_sb, eps_b, b_r0, b_r1,
          v_re, a_re, out_ext, collective, dma_chunks=None):
    for t0, tpc in (dma_chunks or DMA_CHUNKS):
        sl = slice(t0 * D, (t0 + tpc) * D)
        # gpsimd (SWDGE) DMA casts f32 -> bf16 in flight
        nc.gpsimd.dma_start(v_sb[:, sl], v_re[:, sl])
        nc.gpsimd.dma_start(a_sb[:, sl], a_re[:, sl])

    sumv_ps = psum.tile([1, D], F32)
    suma_ps = psum.tile([1, D], F32)
    sync_ps = psum.tile([1, D], F32)
    sumv_sb = small.tile([1, D], F32)

    n_chunks = len(COMPUTE_CHUNKS)
    for c, (t0, tpc) in enumerate(COMPUTE_CHUNKS):
        first = c == 0
        last = c == n_chunks - 1
        # Per-engine sum-of-squares accumulators: ScalarE owns ss_v, DVE
        # owns ss_a. Sharing one tile serializes the engines on WAW order.
        ss_v = small.tile([P, tpc], F32)
        ss_a = small.tile([P, tpc], F32)

        def vsq(j):
            sl = slice((t0 + j) * D, (t0 + j + 1) * D)
            sq_v = scratch.tile([P, D], BF16)
            nc.scalar.activation(
                sq_v[:], v_sb[:, sl], AF.Square, accum_out=ss_v[:, j:j + 1]
            )

        def asq(j):
            sl = slice((t0 + j) * D, (t0 + j + 1) * D)
            # DVE square-reduce via scalar_tensor_tensor accum
            # (InstTensorTensorReduce faults on this HW)
            sq_a = scratch.tile([P, D], BF16)
            nc.vector.scalar_tensor_tensor(
                out=sq_a[:], in0=a_sb[:, sl], scalar=1.0, in1=a_sb[:, sl],
                op0=OP.mult, op1=OP.mult,
                accum_out=ss_a[:, j:j + 1],
            )
            # prod = v*a (bf16 2x mode); its weighted row-sum goes
            # through the PE below, so no per-row dot accum is needed
            prod = scratch.tile([P, D], BF16, tag=f"prod{j}")
            nc.vector.tensor_tensor(
                out=prod[:], in0=v_sb[:, sl], in1=a_sb[:, sl], op=OP.mult
            )
            return prod

        # Batched 1/max(norm, eps) per tensor. The sqrt bias keeps
        # sqrt(0) finite, matching F.normalize's max(norm, 1e-12) for
        # all realizable inputs.
        def vnorm():
            nrm_v = small.tile([P, tpc], F32)
            nc.scalar.activation(nrm_v[:], ss_v[:], AF.Sqrt, bias=eps_b[:])
            inv_v = small.tile([P, tpc], F32)
            nc.vector.reciprocal(inv_v[:], nrm_v[:])
            inv_vb = small.tile([P, tpc], BF16)
            nc.scalar.copy(inv_vb[:], inv_v[:])
            return inv_v, inv_vb

        def anorm(inv_v):
            nrm_a = small.tile([P, tpc], F32)
            nc.scalar.activation(nrm_a[:], ss_a[:], AF.Sqrt, bias=eps_b[:])
            inv_a = small.tile([P, tpc], F32)
            nc.vector.reciprocal(inv_a[:], nrm_a[:])
            inv_ab = small.tile([P, tpc], BF16)
            nc.scalar.copy(inv_ab[:], inv_a[:])
            invva_b = small.tile([P, tpc], BF16)
            nc.vector.tensor_mul(invva_b[:], inv_v[:], inv_a[:])
            return inv_ab, invva_b

        def mm(ps, wcol, rhs, st, sp):
            nc.tensor.matmul(ps[:], lhsT=wcol, rhs=rhs, start=st, stop=sp)

        if not last:
            prods = []
            for j in range(tpc):
                vsq(j)
                prods.append(asq(j))
            inv_v, inv_vb = vnorm()
            inv_ab, invva_b = anorm(inv_v)
            for j in range(tpc):
                sl = slice((t0 + j) * D, (t0 + j + 1) * D)
                st = first and j == 0
                # suma first: its epilogue consumer (PSUM->SBUF copy on
                # ScalarE) can start two matmuls before sync's reduce
                mm(suma_ps, inv_ab[:, j:j + 1], a_sb[:, sl], st, False)
                mm(sumv_ps, inv_vb[:, j:j + 1], v_sb[:, sl], st, False)
                mm(sync_ps, invva_b[:, j:j + 1], prods[j][:], st, False)
        else:
            # Last chunk (1 tile): the entire v-side chain (square, norm,
            # sumv matmul, PSUM->SBUF copy of the finished sumv) is emitted
            # first so in-order engines run it during the final a-tile's
            # DMA; only the a-side chain and two matmuls trail the last
            # byte, and the epilogue dot reads suma from PSUM directly.
            sl = slice(t0 * D, (t0 + tpc) * D)
            vsq(0)
            inv_v, inv_vb = vnorm()
            mm(sumv_ps, inv_vb[:, 0:1], v_sb[:, sl], False, True)
            nc.scalar.copy(sumv_sb[:], sumv_ps[:])
            prod = asq(0)
            inv_ab, invva_b = anorm(inv_v)
            mm(suma_ps, inv_ab[:, 0:1], a_sb[:, sl], False, True)
            mm(sync_ps, invva_b[:, 0:1], prod[:], False, True)

    # ---- epilogue: scalars on partition 0 ----
    # tot = C_ASYNC * <suma (PSUM), sumv_sb>   (DVE; sumv was copied to
    # SBUF during the final a-DMA, so no PSUM->SBUF copy sits here)
    # syn = -(C_SYNC + C_ASYNC) * sum(sync_ps)   (ScalarE, in parallel)
    # diff = tot + syn = async_mean - sync_mean
    # loss/8 = r1 + w*(r0 - r1), r0/r1 relu branches pre-scaled by 1/8
    tot = small.tile([1, 1], F32)
    dum = small.tile([1, D], F32)
    nc.vector.scalar_tensor_tensor(
        out=dum[:], in0=suma_ps[:], scalar=C_ASYNC, in1=sumv_sb[:],
        op0=OP.mult, op1=OP.mult, accum_out=tot[:],
    )
    syn = small.tile([1, 1], F32)
    dum2 = small.tile([1, D], F32)
    nc.scalar.activation(
        dum2[:], sync_ps[:], AF.Copy, scale=-(C_SYNC + C_ASYNC),
        accum_out=syn[:],
    )
    diff = small.tile([1, 1], F32)
    nc.vector.tensor_add(diff[:], tot[:], syn[:])
    # Both margin branches, relu, and the target blend stay on DVE: the
    # tot/diff chain is already there, so no cross-engine hops remain in
    # the tail (each hop costs ~0.15 us of semaphore latency).
    marg = small.tile([1, 2], F32)
    nc.vector.tensor_scalar(
        marg[:, 0:1], diff[:], 1.0 / N_CORES, MARGIN / N_CORES,
        op0=OP.mult, op1=OP.add,
    )
    nc.vector.tensor_scalar(
        marg[:, 1:2], diff[:], -1.0 / N_CORES, MARGIN * 0.1 / N_CORES,
        op0=OP.mult, op1=OP.add,
    )
    relu = small.tile([1, 2], F32)
    nc.vector.tensor_scalar_max(relu[:], marg[:], 0.0)
    d01 = small.tile([1, 1], F32)
    nc.vector.tensor_sub(d01[:], relu[:, 0:1], relu[:, 1:2])
    wd = small.tile([1, 1], F32)
    nc.vector.tensor_mul(wd[:], d01[:], w_sb[:])
    lscaled = small.tile([1, 1], F32)
    nc.vector.tensor_add(lscaled[:], wd[:], relu[:, 1:2])

    if collective:
        loss_bounce = dram.tile([1, 1], F32)
        out_bounce = dram.tile([1, 1], F32)
        nc.gpsimd.dma_start(loss_bounce[:], lscaled[:])
        nc.gpsimd.collective_compute(
            "AllReduce",
            OP.add,
            replica_groups=[list(range(N_CORES))],
            ins=[loss_bounce.opt()],
            outs=[out_bounce.opt()],
        )
        nc.gpsimd.dma_start(out_ext[:], out_bounce[:])
    else:
        nc.sync.dma_start(out_ext[:], lscaled[:])


_NC = None


def _get_nc():
    global _NC
    if _NC is None:
        _NC = _build()
    return _NC


def make_in_maps(visual_features, audio_features, targets):
    vf = np.asarray(visual_features)
    af = np.asarray(audio_features)
    tg = np.asarray(targets)
    return [
        {
            "v": np.ascontiguousarray(vf[i], dtype=np.float32),
            "a": np.ascontiguousarray(af[i], dtype=np.float32),
            "w": np.array([[float(tg[i])]], dtype=np.float32),
        }
        for i in range(N_CORES)
    ]


def kernel(visual_features, audio_features, targets):
    nc = _get_nc()
    in_maps = make_in_maps(visual_features, audio_features, targets)
    res = run_bass_kernel_spmd(nc, in_maps, core_ids=list(range(N_CORES)))
    # Each core's out is its batch element's loss pre-scaled by 1/8; the
    # global mean is the sum of the 8 partials.
    total = np.float32(0.0)
    for i in range(N_CORES):
        total += np.asarray(res.results[i]["out"], dtype=np.float32).reshape(())
    return np.float32(total)


if __name__ == "__main__":
    rng = np.random.default_rng(0)
    v = rng.standard_normal((N_CORES, S, D)).astype(np.float32)
    a = rng.standard_normal((N_CORES, S, D)).astype(np.float32)
    t = rng.integers(0, 2, (N_CORES,)).astype(np.int32)
    print(kernel(visual_features=v, audio_features=a, targets=t))
